# revision 1
# baseline (speedup 1.0000x reference)
"""Causal self-attention on 8 TRN2 NeuronCores (bf16, A/B interleaved).

Problem: x[4, 2048, 1024], w_qkv[3072, 1024], w_proj[1024, 1024],
16 heads x 64 dims, causal softmax attention, output [4, 2048, 1024].

Sharding: core c handles (batch b = c//2, head-group hg = c%2).
Each head-group = 8 heads = 512 channels. Tensor-parallel over heads:
each core computes a *partial* projection output [2048, 1024]; the host
sums the two head-group partials per batch (the "all-reduce" of TP).

Engine balance: PE total ~230us, ACT (exp) total ~190us.  The attention
j-loops are ACT-bound (one [128,1024] exp per key tile), while the QKV
projection (phase A) and output projection are PE-bound with ACT idle.
So phase A computes only the tiles needed by query block 0 up front;
the remaining A tiles and the output projections are emitted as "fill"
work interleaved between attention matmuls, keeping the PE busy while
ACT drains exps.  All matmuls bf16 (full-rate PE, LDW overlap via
row-group tiling on the K=64 S-matmul pairs).

Per-core dataflow:
  Phase A:  QT = Wq @ X^T, KT = Wk @ X^T  [512, 2048] (heads on rows)
            V  = X @ Wv^T [2048, 512] (+ ones column per head)
            8 PSUM-accumulated matmuls per tile; PSUM->SBUF bf16 copies
            on DVE (ACT stays free for exp).
  Phase B (per 512-query block qi, head pair hp, key tile j):
            ST pair = K_h^T Q_h, both heads -> one [128,1024] PSUM
            (diag tiles compute only the valid query range 512-128*o);
            PT = exp(0.125*ST) in ONE ACT op -> bf16; diag staircase
            zeroed by gpsimd affine_select (keep c >= p);
            YT_h += [V_h | 1]^T @ PT_h  (row 64 = denominators).
  Normalize: 1/denoms via DVE reciprocal_approx_fast, R = e8^T @ r
            broadcast matmul, ytu = scratch * R (bf16).
  Proj:     out tile = ytu^T-contracted with w_proj slice; PSUM->SBUF
            copy on DVE, DMA out.  Emitted as fill into the NEXT query
            block's j-loop.
"""

import numpy as np
from contextlib import ExitStack

import concourse.bass as bass
import concourse.tile as tile
from concourse import bacc, mybir
from concourse.bass_utils import run_bass_kernel_spmd

B, T, C, H, D = 4, 2048, 1024, 16, 64
HG = 2                 # head groups (tensor-parallel ways)
HPG = H // HG          # 8 heads per group
CG = HPG * D           # 512 channels per group
P = 128
NQI = T // 512         # 4 query blocks
NJT = T // P           # 16 key tiles
F32 = mybir.dt.float32
F32R = mybir.dt.float32r
BF16 = mybir.dt.bfloat16

# fill closures popped per j-loop iteration, by query block
FILL_PACE = {0: 8, 1: 6, 2: 4, 3: 2}

_CACHE = {}


def _build_core_program():
    nc = bacc.Bacc("TRN2", target_bir_lowering=False, debug=False, num_devices=8)
    xt = nc.dram_tensor("xt", [C, T], BF16, kind="ExternalInput").ap()
    wqkvt = nc.dram_tensor("wqkvt", [C, 3 * CG], BF16, kind="ExternalInput").ap()
    wpt = nc.dram_tensor("wpt", [CG, C], BF16, kind="ExternalInput").ap()
    out = nc.dram_tensor("out", [T, C], F32, kind="ExternalOutput").ap()

    with tile.TileContext(nc) as tc:
        with ExitStack() as ctx:
            _attention(ctx, tc, xt, wqkvt, wpt, out)
    nc.compile()
    return nc


def _attention(ctx, tc, xt, wqkvt, wpt, out):
    nc = tc.nc

    persist = ctx.enter_context(tc.tile_pool(name="persist", bufs=1))
    qt = persist.tile([P, 4, T], BF16, tag="qt")       # QT[c*128+p, i] at [p, c, i]
    kt = persist.tile([P, 4, T], BF16, tag="kt")
    v = persist.tile([P, NJT, HPG * 65], BF16, tag="v")  # [V_h | 1] per key tile
    ytu = persist.tile([P, 4, T], BF16, tag="ytu")     # normalized YT

    consts = ctx.enter_context(tc.tile_pool(name="consts", bufs=1))
    cstage_ctx = ExitStack()
    stage = cstage_ctx.enter_context(tc.tile_pool(name="cstage", bufs=1))
    # E matrices: e8[pc][h, c] = 1 iff chunk-pc channel c belongs to head h
    e8 = []
    for pc in range(4):
        es = stage.tile([8, P], F32, tag="cste", name=f"e8s{pc}")
        nc.gpsimd.memset(es, 0.0)
        e2d = es.rearrange("h (a b) -> h a b", a=2)
        nc.gpsimd.affine_select(
            out=e2d, in_=e2d, compare_op=mybir.AluOpType.not_equal, fill=1.0,
            base=-2 * pc, pattern=[[-1, 2], [0, 64]], channel_multiplier=1,
        )
        e = consts.tile([8, P], F32R, tag=f"e8_{pc}", name=f"e8_{pc}")
        nc.vector.tensor_copy(e, es)
        e8.append(e)
    cstage_ctx.close()
    # ones columns of V (col 64 of each 65-wide head slot); bf16 1.0 = 0x3f80
    v_h = v.rearrange("p j (h e) -> p j h e", e=65)
    nc.gpsimd.memset(v_h[:, :, :, 64:65].bitcast(mybir.dt.uint16), 0x3F80)

    # ---------------- Phase A inputs (persist through the j-loops) --------
    a_x = ctx.enter_context(tc.tile_pool(name="phaseA_x", bufs=1))
    a_w = ctx.enter_context(tc.tile_pool(name="phaseA_w", bufs=1))
    xsb = a_x.tile([P, 8, T], BF16, tag="xsb")
    wsb = a_w.tile([P, 8, 3 * CG], BF16, tag="wsb")
    for g in range(8):
        nc.sync.dma_start(xsb[:, g, :], xt[g * P:(g + 1) * P, :])
        nc.sync.dma_start(wsb[:, g, :], wqkvt[g * P:(g + 1) * P, :])

    wpt_pool = ctx.enter_context(tc.tile_pool(name="wpt", bufs=1))
    wpt_sb = wpt_pool.tile([P, 4, C], BF16, tag="wpt")
    for pc in range(4):
        nc.sync.dma_start(wpt_sb[:, pc, :], wpt[pc * P:(pc + 1) * P, :])

    def emit_qk_tile(psum_pool, m, ib):
        """One QT/KT output tile: 8 accumulated matmuls + DVE copy."""
        dst, mc = (qt, m) if m < 4 else (kt, m - 4)
        wcol = (0 if m < 4 else CG) + mc * P
        ps = psum_pool.tile([P, 512], F32, tag="pj", name="aqk")
        for g in range(8):
            nc.tensor.matmul(
                ps, wsb[:, g, wcol:wcol + P],
                xsb[:, g, ib * 512:(ib + 1) * 512],
                start=(g == 0), stop=(g == 7),
            )
            yield 1
        nc.vector.tensor_copy(dst[:, mc, ib * 512:(ib + 1) * 512], ps)
        yield 1

    def emit_v_tile(psum_pool, it):
        ps = psum_pool.tile([P, 512], F32, tag="pj", name="av")
        for g in range(8):
            nc.tensor.matmul(
                ps, xsb[:, g, it * P:(it + 1) * P],
                wsb[:, g, 2 * CG:3 * CG],
                start=(g == 0), stop=(g == 7),
            )
            yield 1
        nc.vector.tensor_copy(
            v_h[:, it, :, 0:64], ps.rearrange("p (h e) -> p h e", e=64)
        )
        yield 1

    def emit_proj_tile(psum_pool, opool, it, nb):
        ps = psum_pool.tile([P, 512], F32, tag="pj", name="ops")
        for pc in range(4):
            nc.tensor.matmul(
                ps, ytu[:, pc, it * P:(it + 1) * P],
                wpt_sb[:, pc, nb * 512:(nb + 1) * 512],
                start=(pc == 0), stop=(pc == 3),
            )
            yield 1
        osb = opool.tile([P, 512], F32, tag="osb")
        nc.vector.tensor_copy(osb, ps)
        nc.sync.dma_start(
            out[it * P:(it + 1) * P, nb * 512:(nb + 1) * 512], osb
        )
        yield 1

    # ---------------- Upfront: tiles needed by query block 0 ----------------
    with ExitStack() as actx:
        a_psum = actx.enter_context(
            tc.tile_pool(name="phaseA_ps", bufs=4, space="PSUM")
        )
        for m in range(8):
            for _ in emit_qk_tile(a_psum, m, 0):
                pass
        for it in range(4):
            for _ in emit_v_tile(a_psum, it):
                pass

    # ---------------- Phase B pools ----------------
    st_ps = ctx.enter_context(tc.tile_pool(name="st_ps", bufs=2, space="PSUM"))
    yt_ps_pool = ctx.enter_context(tc.tile_pool(name="yt_ps", bufs=1, space="PSUM"))
    pj_ps = ctx.enter_context(tc.tile_pool(name="pj_ps", bufs=2, space="PSUM"))
    pt_pool = ctx.enter_context(tc.tile_pool(name="pt", bufs=3))
    sc_pool = ctx.enter_context(tc.tile_pool(name="sc", bufs=1))
    d_pool = ctx.enter_context(tc.tile_pool(name="d", bufs=2))
    r_pool = ctx.enter_context(tc.tile_pool(name="r", bufs=2))
    o_pool = ctx.enter_context(tc.tile_pool(name="o", bufs=2))

    # Fill stream: remaining A tiles in the order later query blocks need
    # them (block qi needs qt/kt column block ib=qi and V tiles 4qi..4qi+3),
    # tagged with that qi so they can be force-drained before its j-loop
    # (an S-matmul emitted before its A-tile fill would deadlock the PE
    # FIFO on a semaphore only satisfiable by instructions behind it).
    fill = []               # list of [tag, generator]; tag=None for proj
    for ib in range(1, 4):
        for m in range(8):
            fill.append([ib, emit_qk_tile(pj_ps, m, ib)])
        for it in range(4 * ib, 4 * ib + 4):
            fill.append([ib, emit_v_tile(pj_ps, it)])

    def pop_fill(k):
        while k > 0 and fill:
            ent = fill[0]
            if next(ent[1], None) is None:
                fill.pop(0)
            else:
                k -= 1

    def drain_fill(up_to_tag=None):
        i = 0
        while i < len(fill):
            tag, gen = fill[i]
            if up_to_tag is None or (tag is not None and tag <= up_to_tag):
                for _ in gen:
                    pass
                fill.pop(i)
            else:
                i += 1

    # ---------------- Phase B ----------------
    for qi in range(NQI):
        njt = 4 * qi + 4          # key tiles in causal range for this block
        pace = FILL_PACE[qi]
        drain_fill(up_to_tag=qi)  # qt/kt/v this block reads must be emitted
        d_q = d_pool.tile([8, 512], F32, tag="dq")   # denoms, row = head
        scratch = {}              # per-head unnormalized [Y_h; denom]
        for hp in range(4):       # head pairs -> partition rows 0-63 / 64-127
            yt_tiles = [
                yt_ps_pool.tile([65, 512], F32, tag=f"yt{s}", name=f"yt{s}")
                for s in range(2)
            ]
            prev = None           # software pipeline: PV trails S/exp by one
            for j in range(njt):
                o = j - 4 * qi     # diagonal offset (>=0 on causal diagonal)
                off = 128 * o if o > 0 else 0   # first valid query column
                W = 512 - off
                st = st_ps.tile([P, 1024], F32, tag="st")
                for s in range(2):
                    r0 = s * 64
                    nc.tensor.matmul(
                        st[:, s * 512 + off:(s + 1) * 512],
                        kt[r0:r0 + 64, hp, j * P:(j + 1) * P],
                        qt[r0:r0 + 64, hp, qi * 512 + off:(qi + 1) * 512],
                        start=True, stop=True,
                    )
                pt = pt_pool.tile([P, 1024], BF16, tag="pt")
                st3 = st.rearrange("p (s q) -> p s q", s=2)[:, :, off:]
                pt3 = pt.rearrange("p (s q) -> p s q", s=2)[:, :, off:]
                nc.scalar.activation(
                    pt3, st3, mybir.ActivationFunctionType.Exp, scale=0.125
                )
                if o >= 0:
                    # zero the still-invalid staircase: keep where q-col >= p
                    nc.gpsimd.affine_select(
                        out=pt3, in_=pt3, compare_op=mybir.AluOpType.is_ge,
                        fill=0.0, base=0, pattern=[[0, 2], [1, W]],
                        channel_multiplier=-1,
                    )
                pop_fill(pace)
                if prev is not None:
                    _emit_pv(nc, v, yt_tiles, prev, hp, njt)
                prev = (j, off, pt)
            _emit_pv(nc, v, yt_tiles, prev, hp, njt)

            for s in range(2):
                h = 2 * hp + s
                # unnormalized [Y_h; denom] -> SBUF scratch, then DMA the
                # denom row into d_q (DMA writes any partition; engines
                # can only address 32-aligned partition bases)
                sc = sc_pool.tile([65, 512], F32R, tag=f"sc{h}", name=f"sc{h}")
                nc.vector.tensor_copy(sc, yt_tiles[s][:, :])
                nc.sync.dma_start(d_q[h:h + 1, :], sc[64:65, :].bitcast(F32))
                scratch[h] = sc

        # normalize this query block: R = e8^T @ (1/denoms)
        r_q = r_pool.tile([8, 512], F32, tag="rq")
        nc.vector.reciprocal_approx_fast(out=r_q, in_=d_q)
        r_qr = r_pool.tile([8, 512], F32R, tag="rqr")
        nc.vector.tensor_copy(r_qr, r_q)
        for pc in range(4):
            rps = pj_ps.tile([P, 512], F32, tag="pj", name="rps")
            nc.tensor.matmul(rps, e8[pc], r_qr, start=True, stop=True)
            for s in range(2):
                h = 2 * pc + s
                nc.vector.tensor_mul(
                    out=ytu[s * 64:s * 64 + 64, pc, qi * 512:(qi + 1) * 512],
                    in0=scratch[h][0:64, :],
                    in1=rps[s * 64:s * 64 + 64, :],
                )

        # this block's projections become fill for the next block's j-loop
        projs = [
            emit_proj_tile(pj_ps, o_pool, it, nb)
            for it in range(4 * qi, 4 * qi + 4) for nb in range(2)
        ]
        if qi < NQI - 1:
            fill.extend([None, gen] for gen in projs)
        else:
            drain_fill()
            for gen in projs:
                for _ in gen:
                    pass
    drain_fill()


def _emit_pv(nc, v, yt_tiles, prev, hp, njt):
    j, off, pt = prev
    for s in range(2):
        h = 2 * hp + s
        nc.tensor.matmul(
            yt_tiles[s][:, off:512],
            v[:, j, h * 65:(h + 1) * 65],
            pt[:, s * 512 + off:(s + 1) * 512],
            start=(j == 0), stop=(j == njt - 1),
        )


def _prep_inputs(x, w_qkv, w_proj):
    """Build the 8 per-core input maps (host-side sharding + transposes)."""
    import ml_dtypes
    bf16 = ml_dtypes.bfloat16
    xts = [np.ascontiguousarray(x[b].T).astype(bf16) for b in range(B)]
    wqkvts, wpts = [], []
    for hg in range(HG):
        s = hg * CG
        wq = w_qkv[s:s + CG]
        wk = w_qkv[C + s:C + s + CG]
        wv = w_qkv[2 * C + s:2 * C + s + CG]
        wqkvts.append(
            np.ascontiguousarray(np.concatenate([wq, wk, wv], 0).T).astype(bf16)
        )
        wpts.append(np.ascontiguousarray(w_proj[:, s:s + CG].T).astype(bf16))
    in_maps = []
    for c in range(8):
        b, hg = c // 2, c % 2
        in_maps.append({"xt": xts[b], "wqkvt": wqkvts[hg], "wpt": wpts[hg]})
    return in_maps


def kernel(x, w_qkv, w_proj):
    x = np.asarray(x, dtype=np.float32)
    w_qkv = np.asarray(w_qkv, dtype=np.float32)
    w_proj = np.asarray(w_proj, dtype=np.float32)

    if "nc" not in _CACHE:
        _CACHE["nc"] = _build_core_program()
    nc = _CACHE["nc"]

    in_maps = _prep_inputs(x, w_qkv, w_proj)
    res = run_bass_kernel_spmd(nc, in_maps, core_ids=list(range(8)))
    outs = [r["out"] for r in res.results]
    full = np.empty((B, T, C), dtype=np.float32)
    for b in range(B):
        full[b] = outs[2 * b] + outs[2 * b + 1]
    return full



# revision 2
# speedup vs baseline: 1.0157x; 1.0157x over previous
"""Causal self-attention on 8 TRN2 NeuronCores (bf16 + fp8-DR QK phase).

Problem: x[4, 2048, 1024], w_qkv[3072, 1024], w_proj[1024, 1024],
16 heads x 64 dims, causal softmax attention, output [4, 2048, 1024].

Sharding: core c handles (batch b = c//2, head-group hg = c%2); the host
sums the two head-group fp16 partials per batch in f32 (TP all-reduce).

Measured HW model driving the design: PE matmul streams 1 moving
column/cycle @2.4GHz regardless of dtype; fp8 DoubleRow streams two
k-tiles at 2 cols/cycle (half the instructions per contraction);
row-group-disjoint matmul tiles execute CONCURRENTLY; ACT exp costs
free-size cols @1.2GHz + ~200ns/op; DMA is descriptor-bound (~100GB/s
per queue with >=2KB descriptors; only sync/scalar/gpsimd can issue).

Schedule (ACT ~159us busy, PE stream ~225us -> PE-bound, exec ~265us):
  - Q/K projection tiles: 4 naive fp8e4m3 DoubleRow matmuls (K=256
    each); Q,K noise (~8.5%) survives the softmax at ~9e-3 final rel
    err (gate 2e-2). V and the output projection stay bf16 (fp8 there
    breaks the error budget; compensated fp8 is slower than bf16).
  - Flat (qi, hp, j) software pipeline: S for step n+1 is pre-issued
    right after exp(n); PV trails 6 steps (9-deep pt pool) so PV waits
    never block the in-order PE queue; phase-A and projection tiles
    interleave as paced fill in exp's shadow, (qi, hp)-tagged and
    force-drained just before first use.
  - Per-head-pair normalization (denominator row DMA'd to a [2,512]
    tile, reciprocal + e2 broadcast matmul + ytu scale) overlaps the
    next pair's j-loop; block qi's projection becomes fill for qi+1.
  - Host packs every input in its exact SBUF layout (x token-block-
    major, fp8 copy for QK / bf16 for V) so each load is one large-
    descriptor DMA streamed in first-use order across 3 queues.
  - Block-3 projections run at the tail from dedicated 4-deep PSUM/
    output pools (phase-B pools closed first); fp16 output halves the
    write traffic, host does the final f32 sum.
"""

import numpy as np
from contextlib import ExitStack

import concourse.bass as bass
import concourse.tile as tile
from concourse import bacc, mybir
from concourse.bass_utils import run_bass_kernel_spmd

B, T, C, H, D = 4, 2048, 1024, 16, 64
HG = 2                 # head groups (tensor-parallel ways)
HPG = H // HG          # 8 heads per group
CG = HPG * D           # 512 channels per group
P = 128
NQI = T // 512         # 4 query blocks
NJT = T // P           # 16 key tiles
F32 = mybir.dt.float32
F32R = mybir.dt.float32r
F16 = mybir.dt.float16
BF16 = mybir.dt.bfloat16
FP8 = mybir.dt.float8e4
DR = mybir.MatmulPerfMode.DoubleRow
WS = 64.0              # host-side Q/K weight scale before fp8 quantization
EXP_SCALE = 0.125 / (WS * WS)

# fill matmuls popped per j-loop iteration, by query block
FILL_PACE = {0: 6, 1: 5, 2: 4, 3: 3}

_CACHE = {}


def _build_core_program():
    # All inputs arrive pre-packed by the host in their exact SBUF layout
    # ([partition, ...free...], contiguous) so every load is one DMA with
    # maximal (4-16KB) descriptors at full HBM bandwidth.
    nc = bacc.Bacc("TRN2", target_bir_lowering=False, debug=False, num_devices=8)
    # both x copies are packed token-block-major so the first tiles' slices
    # arrive first and every transfer keeps >=2KB descriptors
    xq = nc.dram_tensor("xq", [P, 4, 8, 512], FP8, kind="ExternalInput").ap()
    # xt is packed token-block-major so each V tile's slice is one small
    # contiguous DMA streamed in need-order
    xt = nc.dram_tensor("xt", [P, NJT, 8, P], BF16, kind="ExternalInput").ap()
    wqk = nc.dram_tensor("wqk", [P, 8, 2 * CG], FP8, kind="ExternalInput").ap()
    wv = nc.dram_tensor("wv", [P, 8, CG], BF16, kind="ExternalInput").ap()
    wpt = nc.dram_tensor("wpt", [P, 4, C], BF16, kind="ExternalInput").ap()
    # fp16 partials, [partition, token-block, channel]; host un-shuffles
    out = nc.dram_tensor("out", [P, NJT, C], F16, kind="ExternalOutput").ap()

    with tile.TileContext(nc) as tc:
        with ExitStack() as ctx:
            _attention(ctx, tc, xq, xt, wqk, wv, wpt, out)
    nc.compile()
    return nc


def _attention(ctx, tc, xq, xt, wqk, wv, wpt, out):
    nc = tc.nc

    persist = ctx.enter_context(tc.tile_pool(name="persist", bufs=1))
    qt = persist.tile([P, 4, T], BF16, tag="qt")       # QT'[c*128+p, i] at [p, c, i]
    kt = persist.tile([P, 4, T], BF16, tag="kt")
    v = persist.tile([P, NJT, HPG * 65], BF16, tag="v")  # [V_h | 1] per key tile
    ytu = persist.tile([P, 4, T], BF16, tag="ytu")     # normalized YT

    consts = ctx.enter_context(tc.tile_pool(name="consts", bufs=1))
    cstage_ctx = ExitStack()
    stage = cstage_ctx.enter_context(tc.tile_pool(name="cstage", bufs=1))
    # e2[s, c] = 1 iff channel c (of a 128-chunk) belongs to head-slot s
    e2s = stage.tile([2, P], F32, tag="cste", name="e2s")
    nc.gpsimd.memset(e2s, 0.0)
    e2d = e2s.rearrange("s (a b) -> s a b", a=2)
    nc.gpsimd.affine_select(
        out=e2d, in_=e2d, compare_op=mybir.AluOpType.not_equal, fill=1.0,
        base=0, pattern=[[-1, 2], [0, 64]], channel_multiplier=1,
    )
    e2 = consts.tile([2, P], F32R, tag="e2", name="e2")
    nc.vector.tensor_copy(e2, e2s)
    cstage_ctx.close()
    # ones columns of V (col 64 of each 65-wide head slot); bf16 1.0 = 0x3f80
    v_h = v.rearrange("p j (h e) -> p j h e", e=65)
    nc.gpsimd.memset(v_h[:, :, :, 64:65].bitcast(mybir.dt.uint16), 0x3F80)

    # ---------------- Phase A inputs ----------------
    # DMA cost is descriptor-count bound (~contiguous runs, one engine per
    # transfer), so: full-chunk transfers only (1-4KB descriptors), spread
    # across four queues, in first-use order.
    a_x = ctx.enter_context(tc.tile_pool(name="phaseA_x", bufs=1))
    a_w = ctx.enter_context(tc.tile_pool(name="phaseA_w", bufs=1))
    xqsb = a_x.tile([P, 4, 8, 512], FP8, tag="xqsb")
    xsb = a_x.tile([P, NJT, 8, P], BF16, tag="xsb")
    wqksb = a_w.tile([P, 8, 2 * CG], FP8, tag="wqksb")
    wvsb = a_w.tile([P, 8, CG], BF16, tag="wvsb")
    wpt_pool = ctx.enter_context(tc.tile_pool(name="wpt", bufs=1))
    wpt_sb = wpt_pool.tile([P, 4, C], BF16, tag="wpt")

    # first-exp path = wqk + xq block 0 (1.5MB) and first-PV path = wv +
    # xt0 (1.25MB) land in parallel across the three queues; the rest
    # streams behind in need-order
    nc.sync.dma_start(wqksb, wqk)
    nc.scalar.dma_start(xqsb[:, 0], xq[:, 0])
    nc.gpsimd.dma_start(wvsb, wv)
    nc.sync.dma_start(xsb[:, 0], xt[:, 0])
    for it in (1, 2, 3):
        q = (None, nc.gpsimd, nc.scalar, nc.gpsimd)[it]
        q.dma_start(xsb[:, it], xt[:, it])
    nc.scalar.dma_start(xqsb[:, 1], xq[:, 1])
    nc.scalar.dma_start(xqsb[:, 2], xq[:, 2])
    nc.gpsimd.dma_start(xqsb[:, 3], xq[:, 3])
    nc.sync.dma_start(wpt_sb, wpt)
    for it in range(4, 16):
        q = nc.scalar if it < 10 else nc.gpsimd
        q.dma_start(xsb[:, it], xt[:, it])

    def emit_qk_tile(psum_pool, m, ib):
        """One QT/KT output tile: 4 fp8 DoubleRow matmuls + DVE copy."""
        dst, mc = (qt, m) if m < 4 else (kt, m - 4)
        wcol = (0 if m < 4 else CG) + mc * P
        ps = psum_pool.tile([P, 512], F32, tag="pj", name="aqk")
        for g in range(4):
            nc.tensor.matmul(
                ps, wqksb[:, 2 * g:2 * g + 2, wcol:wcol + P],
                xqsb[:, ib, 2 * g:2 * g + 2, :],
                start=(g == 0), stop=(g == 3), perf_mode=DR,
            )
            yield 1
        nc.vector.tensor_copy(dst[:, mc, ib * 512:(ib + 1) * 512], ps)
        yield 1

    def emit_v_tile(psum_pool, it):
        ps = psum_pool.tile([P, 512], F32, tag="pj", name="av")
        for g in range(8):
            nc.tensor.matmul(
                ps, xsb[:, it, g, :],
                wvsb[:, g, :],
                start=(g == 0), stop=(g == 7),
            )
            yield 1
        nc.vector.tensor_copy(
            v_h[:, it, :, 0:64], ps.rearrange("p (h e) -> p h e", e=64)
        )
        yield 1

    def emit_proj_tile(psum_pool, opool, it, nb):
        ps = psum_pool.tile([P, 512], F32, tag="pj", name="ops")
        for pc in range(4):
            nc.tensor.matmul(
                ps, ytu[:, pc, it * P:(it + 1) * P],
                wpt_sb[:, pc, nb * 512:(nb + 1) * 512],
                start=(pc == 0), stop=(pc == 3),
            )
            yield 1
        osb = opool.tile([P, 512], F16, tag="osb")
        nc.vector.tensor_copy(osb, ps)
        nc.sync.dma_start(out[:, it, nb * 512:(nb + 1) * 512], osb)
        yield 1

    # ---------------- Upfront: just enough for the first exp --------------
    with ExitStack() as actx:
        a_psum = actx.enter_context(
            tc.tile_pool(name="phaseA_ps", bufs=2, space="PSUM")
        )
        for m in (0, 4):
            for _ in emit_qk_tile(a_psum, m, 0):
                pass

    with ExitStack() as bctx:
        st_ps = bctx.enter_context(tc.tile_pool(name="st_ps", bufs=2, space="PSUM"))
        yt_ps_pool = bctx.enter_context(tc.tile_pool(name="yt_ps", bufs=1, space="PSUM"))
        pj_ps = bctx.enter_context(tc.tile_pool(name="pj_ps", bufs=2, space="PSUM"))
        pt_pool = bctx.enter_context(tc.tile_pool(name="pt", bufs=9))
        sc_pool = bctx.enter_context(tc.tile_pool(name="sc", bufs=1))
        d_pool = bctx.enter_context(tc.tile_pool(name="d", bufs=2))
        r_pool = bctx.enter_context(tc.tile_pool(name="r", bufs=2))
        o_pool = bctx.enter_context(tc.tile_pool(name="o", bufs=2))

        # Fill stream: remaining A tiles tagged (qi, hp) = first reader;
        # the drain before S(qi, hp, 0) guarantees availability, pace pops
        # keep it flowing in exp's shadow. Proj tiles untagged (any time).
        fill = []           # list of [tag, generator]; tag=None for proj
        for it in range(4):
            fill.append([(0, 1), emit_v_tile(pj_ps, it)])
        for hp in (1, 2, 3):
            fill.append([(0, hp), emit_qk_tile(pj_ps, hp, 0)])
            fill.append([(0, hp), emit_qk_tile(pj_ps, hp + 4, 0)])
        for ib in range(1, 4):
            fill.append([(ib, 0), emit_qk_tile(pj_ps, 0, ib)])
            fill.append([(ib, 0), emit_qk_tile(pj_ps, 4, ib)])
            fill.append([(ib, 0), emit_v_tile(pj_ps, 4 * ib)])
            for it in range(4 * ib + 1, 4 * ib + 4):
                fill.append([(ib, 1), emit_v_tile(pj_ps, it)])
            for hp in (1, 2, 3):
                fill.append([(ib, hp), emit_qk_tile(pj_ps, hp, ib)])
                fill.append([(ib, hp), emit_qk_tile(pj_ps, hp + 4, ib)])

        def pop_fill(k):
            while k > 0 and fill:
                ent = fill[0]
                if next(ent[1], None) is None:
                    fill.pop(0)
                else:
                    k -= 1

        def drain_fill(up_to_tag=None):
            i = 0
            while i < len(fill):
                tag, gen = fill[i]
                if up_to_tag is None or (tag is not None and tag <= up_to_tag):
                    for _ in gen:
                        pass
                    fill.pop(i)
                else:
                    i += 1

        # ------------- flat (qi, hp, j) software pipeline -----------------
        steps = []
        for qi in range(NQI):
            njt = 4 * qi + 4
            for hp in range(4):
                for j in range(njt):
                    steps.append((qi, hp, j))

        yt_by = {}
        scratch = {}

        def emit_s(qi, hp, j):
            if j == 0:
                drain_fill(up_to_tag=(qi, hp))
                yt_by[(qi, hp)] = [
                    yt_ps_pool.tile([65, 512], F32, tag=f"yt{s}", name=f"yt{s}")
                    for s in range(2)
                ]
            o = j - 4 * qi
            off = 128 * o if o > 0 else 0
            st = st_ps.tile([P, 1024], F32, tag="st")
            for s in range(2):
                r0 = s * 64
                nc.tensor.matmul(
                    st[:, s * 512 + off:(s + 1) * 512],
                    kt[r0:r0 + 64, hp, j * P:(j + 1) * P],
                    qt[r0:r0 + 64, hp, qi * 512 + off:(qi + 1) * 512],
                    start=True, stop=True,
                )
            return st, off

        def finalize_hp(qi, hp):
            """Scratch copies + per-pair normalization; overlaps the next
            pair's j-loop. ytu chunk pc == hp."""
            yt_tiles = yt_by.pop((qi, hp))
            d2 = d_pool.tile([2, 512], F32, tag="d", name=f"d{hp}")
            for s in range(2):
                h = 2 * hp + s
                sc = sc_pool.tile([65, 512], F32R, tag=f"sc{h}", name=f"sc{h}")
                nc.vector.tensor_copy(sc, yt_tiles[s][:, :])
                nc.gpsimd.dma_start(d2[s:s + 1, :], sc[64:65, :].bitcast(F32))
                scratch[h] = sc
            r2 = r_pool.tile([2, 512], F32, tag="r2", name=f"r{hp}")
            nc.vector.reciprocal_approx_fast(out=r2, in_=d2)
            r2r = r_pool.tile([2, 512], F32R, tag="rr", name=f"rr{hp}")
            nc.vector.tensor_copy(r2r, r2)
            rps = pj_ps.tile([P, 512], F32, tag="pj", name="rps")
            nc.tensor.matmul(rps, e2, r2r, start=True, stop=True)
            for s in range(2):
                h = 2 * hp + s
                nc.vector.tensor_mul(
                    out=ytu[s * 64:s * 64 + 64, hp, qi * 512:(qi + 1) * 512],
                    in0=scratch[h][0:64, :],
                    in1=rps[s * 64:s * 64 + 64, :],
                )
            if hp == 3 and qi < NQI - 1:
                fill.extend(
                    [None, emit_proj_tile(pj_ps, o_pool, it, nb)]
                    for it in range(4 * qi, 4 * qi + 4) for nb in range(2)
                )

        pv_pending = []  # (qi, hp, j, off, pt, is_last_j); PV trails 6 steps
        cur_s = emit_s(*steps[0])
        for n, (qi, hp, j) in enumerate(steps):
            njt = 4 * qi + 4
            st, off = cur_s
            pt = pt_pool.tile([P, 1024], BF16, tag="pt")
            st3 = st.rearrange("p (s q) -> p s q", s=2)[:, :, off:]
            pt3 = pt.rearrange("p (s q) -> p s q", s=2)[:, :, off:]
            nc.scalar.activation(
                pt3, st3, mybir.ActivationFunctionType.Exp, scale=EXP_SCALE
            )
            o = j - 4 * qi
            if o >= 0:
                # zero the invalid staircase (128 cols past the diagonal)
                ptb = pt.rearrange("p (s q) -> p s q", s=2)[:, :, off:off + P]
                nc.gpsimd.affine_select(
                    out=ptb, in_=ptb, compare_op=mybir.AluOpType.is_ge,
                    fill=0.0, base=0, pattern=[[0, 2], [1, P]],
                    channel_multiplier=-1,
                )
            # pre-issue the next step's S pair so the next exp is never
            # gated by fills or PV below
            if n + 1 < len(steps):
                cur_s = emit_s(*steps[n + 1])
            pv_pending.append((qi, hp, j, off, pt, j == njt - 1))
            if len(pv_pending) > 6:
                ent = pv_pending.pop(0)
                _emit_pv(nc, v, yt_by, scratch, ent)
                if ent[5]:
                    finalize_hp(ent[0], ent[1])
            # no fills in the first few steps: their operands are still in
            # flight and a waiting fill matmul would block the PE queue
            if n >= 4:
                pop_fill(FILL_PACE[qi])
        for ent in pv_pending:
            _emit_pv(nc, v, yt_by, scratch, ent)
            if ent[5]:
                finalize_hp(ent[0], ent[1])
        drain_fill()

    # ---------------- Tail: block-3 projections with deep pools -----------
    with ExitStack() as tctx:
        tail_ps = tctx.enter_context(
            tc.tile_pool(name="tail_ps", bufs=4, space="PSUM")
        )
        tail_o = tctx.enter_context(tc.tile_pool(name="tail_o", bufs=4))
        for it in range(12, 16):
            for nb in range(2):
                for _ in emit_proj_tile(tail_ps, tail_o, it, nb):
                    pass


def _emit_pv(nc, v, yt_by, scratch, prev):
    qi, hp, j, off, pt, _ = prev
    njt = 4 * qi + 4
    yt_tiles = yt_by[(qi, hp)]
    for s in range(2):
        h = 2 * hp + s
        nc.tensor.matmul(
            yt_tiles[s][:, off:512],
            v[:, j, h * 65:(h + 1) * 65],
            pt[:, s * 512 + off:(s + 1) * 512],
            start=(j == 0), stop=(j == njt - 1),
        )


def _sbufify(a):
    """[C_in, free] -> packed SBUF layout [128, C_in//128, free]."""
    r, f = a.shape
    return np.ascontiguousarray(a.reshape(r // P, P, f).transpose(1, 0, 2))


def _prep_inputs(x, w_qkv, w_proj):
    """Build the 8 per-core input maps: host-side sharding, transposes,
    fp8 quantization, and packing into exact SBUF layouts (so each load
    is a single full-bandwidth DMA)."""
    import ml_dtypes
    bf16 = ml_dtypes.bfloat16
    e4 = ml_dtypes.float8_e4m3

    xts, xqs = [], []
    for b in range(B):
        xtb = np.ascontiguousarray(x[b].T)
        # [p, it, g, tok] token-block-major packing for the V path
        xts.append(np.ascontiguousarray(
            xtb.astype(bf16).reshape(8, P, NJT, P).transpose(1, 2, 0, 3)
        ))
        xqs.append(np.ascontiguousarray(
            xtb.astype(e4).reshape(8, P, 4, 512).transpose(1, 2, 0, 3)
        ))

    wqks, wvs, wpts = [], [], []
    for hg in range(HG):
        s = hg * CG
        wq = w_qkv[s:s + CG]
        wk = w_qkv[C + s:C + s + CG]
        wv_ = w_qkv[2 * C + s:2 * C + s + CG]
        wqks.append(_sbufify(
            np.ascontiguousarray((WS * np.concatenate([wq, wk], 0)).T).astype(e4)
        ))
        wvs.append(_sbufify(np.ascontiguousarray(wv_.T).astype(bf16)))
        wpts.append(_sbufify(
            np.ascontiguousarray(w_proj[:, s:s + CG].T).astype(bf16)
        ))
    in_maps = []
    for c in range(8):
        b, hg = c // 2, c % 2
        in_maps.append({
            "xq": xqs[b], "xt": xts[b],
            "wqk": wqks[hg], "wv": wvs[hg], "wpt": wpts[hg],
        })
    return in_maps


def kernel(x, w_qkv, w_proj):
    x = np.asarray(x, dtype=np.float32)
    w_qkv = np.asarray(w_qkv, dtype=np.float32)
    w_proj = np.asarray(w_proj, dtype=np.float32)

    if "nc" not in _CACHE:
        _CACHE["nc"] = _build_core_program()
    nc = _CACHE["nc"]

    in_maps = _prep_inputs(x, w_qkv, w_proj)
    res = run_bass_kernel_spmd(nc, in_maps, core_ids=list(range(8)))
    # out is [p, token_block, c] fp16; un-shuffle to [T, C] and sum the
    # two head-group partials per batch in f32
    outs = [
        r["out"].astype(np.float32).transpose(1, 0, 2).reshape(T, C)
        for r in res.results
    ]
    full = np.empty((B, T, C), dtype=np.float32)
    for b in range(B):
        full[b] = outs[2 * b] + outs[2 * b + 1]
    return full
